# revision 54
# baseline (speedup 1.0000x reference)
"""MLA-style attention-score kernel for Trainium2 (8 NeuronCores, SPMD).

Computes, for full inputs
    q_nope_with_k_up [B,H,S,r], compressed_kv [B,S,r], rope_attention [B,H,S,S],
    mask [B,S], have_causal_mask scalar:

    nope   = einsum("bhqr,bkr->bhqk", q, kv)
    scores = (rope + nope) / sqrt(192)   (+ causal / padding masks)
    attn   = softmax(scores, -1)
    out    = einsum("bhqk,bkr->bhqr", attn, kv)        [B,H,S,r] fp32

Formulation: exp(s*(rope+nope)) = exp(s*rope) * exp(s*nope).  The host
precomputes E = exp(s*rope) (with causal/padding masks folded in as exact
zeros), pre-transposes q -> qT [r,S] and kv -> kvT [r,S], and casts all
operands to bf16.  On-chip, scores are built TRANSPOSED ([k,q] blocks:
stationary kvT, moving qT) so the exp output is already [k,q] and feeds the
AV matmul as stationary with no PE transpose anywhere.  Row sums come from
1-column ones-matmuls; output is scaled by 1/sum and stored bf16.

Sharding: B*H = 64 head-slots, 8 per core; core c owns batch c//2, heads
(c%2)*8..+8, with that batch's compressed_kv replicated on the core.
"""

import math
import os
import sys

import numpy as np
import ml_dtypes

for _p in ("/opt/trn_rl_repo", "/root/.axon_site/_ro/trn_rl_repo"):
    if os.path.isdir(_p) and _p not in sys.path:
        sys.path.insert(0, _p)

import concourse.bass as bass
import concourse.mybir as mybir
import concourse.tile as tile
from concourse import bass_utils
from concourse.vector_clock import ScopedClock

B, H, S, R = 4, 16, 1024, 512
N_CORES = 8
HPC = (B * H) // N_CORES          # heads per core
P = 128                           # partition block
NSB = S // P                      # 8 s-blocks
NRB = R // P                      # 4 r-chunks
SCALE = 1.0 / math.sqrt(64 + 128)
F32 = mybir.dt.float32
BF16 = mybir.dt.bfloat16
AF = mybir.ActivationFunctionType
BF16NP = ml_dtypes.bfloat16

N_WARM = int(os.environ.get("ATTN_WARM", "30"))   # PE p-state warmup matmuls


class _TC(tile.TileContext):
    """TileContext whose end-of-kernel drain splits its semaphore waits
    across preceding NOPs (walrus in this image rejects >2 sync waits on
    one Drain)."""

    MAX_WAITS = 1

    def _drain_and_barrier(self, tick_clock, wait_clock):
        nop = self.nc.sync.nop(nofuse=True)
        wait_clock.add_sem_waits(
            nop.ins, ScopedClock({None: tick_clock.global_clock})
        )
        si = nop.ins.sync_info
        waits = list(si.on_wait) if si is not None else []
        if len(waits) > self.MAX_WAITS:
            nop.ins.sync_info = mybir.SyncInfo(
                on_wait=waits[: self.MAX_WAITS], on_update=[]
            )
            for i in range(self.MAX_WAITS, len(waits), self.MAX_WAITS):
                extra = self.nc.sync.nop(nofuse=True)
                extra.ins.sync_info = mybir.SyncInfo(
                    on_wait=waits[i : i + self.MAX_WAITS], on_update=[]
                )
        self.nc.sync.drain()
        self.nc.all_engine_barrier()
        popped = self.nc._tile_sem_poison_stack.pop()
        assert popped is self._sem_poison
        # skip the hardware semaphore-clear pass + second barrier (~5us of
        # teardown): the program ends here and nothing re-reads the sems.
        # Do only the python-side bookkeeping clear_and_free_semaphores does.
        sem_nums = [
            s.num if hasattr(s, "num") else s
            for s in self.sems.allocated().values()
        ]
        if sem_nums:
            self.nc._state.prepend_free_semaphores(sem_nums)
            for poison_set in self.nc._tile_sem_poison_stack:
                poison_set.update(sem_nums)


def _legalize_sync_waits(nc, max_waits=1):
    """walrus in this image allows only `max_waits` sync waits per
    instruction: move excess waits onto NOPs inserted just before the
    instruction on the same engine queue."""
    nid = 0
    for f in nc.m.functions:
        for blk in f.blocks:
            insts = blk.instructions
            out = []
            changed = False
            for inst in insts:
                si = inst.sync_info
                if si is not None and len(si.on_wait) > max_waits:
                    waits = list(si.on_wait)
                    n_pre = len(waits) - max_waits
                    for i in range(0, n_pre, max_waits):
                        nop = mybir.InstNoOp(
                            name=f"waitsplit_{nid}", ins=[], outs=[],
                            engine=inst.engine, bass_nofuse=True,
                            sync_info=mybir.SyncInfo(
                                on_wait=waits[i : min(i + max_waits, n_pre)],
                                on_update=[],
                            ),
                        )
                        nid += 1
                        out.append(nop)
                    inst.sync_info = mybir.SyncInfo(
                        on_wait=waits[n_pre:], on_update=list(si.on_update)
                    )
                    changed = True
                out.append(inst)
            if changed:
                blk.instructions = out


def build_program(causal: bool, hpc: int = HPC, legalize: bool = True):
    # qt/kvt arrive pre-shuffled to the SBUF tile layout [block, p, rb*P]
    # so every DMA is 128 descriptors of contiguous 1KB per partition
    nc = bass.Bass()
    qt_d = nc.dram_tensor("qt", [hpc, NSB, P, R], BF16, kind="ExternalInput")
    e_d = nc.dram_tensor("e", [hpc, S, S], BF16, kind="ExternalInput")
    kv_d = nc.dram_tensor("kv", [S, R], BF16, kind="ExternalInput")
    kvt_d = nc.dram_tensor("kvt", [NSB, P, R], BF16, kind="ExternalInput")
    out_d = nc.dram_tensor("out", [hpc, S, R], BF16, kind="ExternalOutput")

    # qb ascending per head, except the LAST head runs qb descending so the
    # kernel's final step is the small nk=1 one (short pipeline tail)
    steps = []
    for h in range(hpc):
        qbs = range(NSB) if h < hpc - 1 else range(NSB - 1, -1, -1)
        steps += [(h, qb) for qb in qbs]

    with _TC(nc) as tc:
        with (
            tc.tile_pool(name="const", bufs=1) as const_pool,
            tc.tile_pool(name="kvp", bufs=1) as kv_pool,
            tc.tile_pool(name="qtp", bufs=2) as qt_pool,
            tc.tile_pool(name="ep", bufs=3) as e_pool,
            tc.tile_pool(name="enp", bufs=3) as en_pool,
            tc.tile_pool(name="atp", bufs=4) as at_pool,
            tc.tile_pool(name="ohp", bufs=4) as oh_pool,
            tc.tile_pool(name="rcp", bufs=4) as rc_pool,
            tc.tile_pool(name="psc", bufs=2, space="PSUM") as sc_pool,
            tc.tile_pool(name="pav", bufs=3, space="PSUM") as av_pool,
            tc.tile_pool(name="psum_s", bufs=1, space="PSUM") as sums_pool,
        ):
            # ---- PE warmup (p-state ramp) on memset tiles, overlapping the
            # prologue DMAs; writes a scratch psum tile nothing reads.
            # Memsets run on DVE so they don't delay the gpsimd SWDGE queue.
            wstat = const_pool.tile([P, P], BF16, tag="wstat")
            wmov = const_pool.tile([P, 512], BF16, tag="wmov")
            ones = const_pool.tile([P, 1], BF16, tag="ones")
            nc.vector.memset(wstat[:], 0.0)
            nc.vector.memset(wmov[:], 0.0)
            nc.vector.memset(ones[:], 1.0)
            warm = av_pool.tile([P, R], F32, tag="av", name="warm_ps")
            for _ in range(N_WARM):
                nc.tensor.matmul(warm[:], wstat[:], wmov[:], start=True,
                                 stop=True)

            # ---- loads. HWDGE rings generate descriptors at ~14ns/desc, so
            # big many-descriptor loads go through SWDGE (gpsimd, 0.34ns/desc):
            # prologue kvt/qt0/kv, the small-kb halves of E, and out stores.
            # Steady-state qt goes on the otherwise-idle ACT ring, the big-kb
            # halves of E on the SP ring.
            kvt = kv_pool.tile([P, NSB, R], BF16, tag="kvt")
            kv = kv_pool.tile([P, NSB, R], BF16, tag="kv")

            heads = {}

            def qt_load(h, eng):
                qt = qt_pool.tile([P, NSB, R], BF16, tag="qt", name="qt_t")
                eng.dma_start(qt[:], qt_d[h].rearrange("n p s -> p n s"))
                return qt

            def e_load(h, kbs, et=None):
                if et is None:
                    et = e_pool.tile([P, NSB, S], BF16, tag="e", name="e_t")
                for kb in kbs:
                    q0 = kb * P if causal else 0
                    eng = nc.sync if kb < NSB // 2 else nc.gpsimd
                    eng.dma_start(
                        et[:, kb, q0:S],
                        e_d[h, kb * P : (kb + 1) * P, q0:S],
                    )
                return et

            def head_load(h, prologue=False):
                if prologue:
                    # progressive block loads: step (0,0) waits only on the
                    # kb=0/qb=0 blocks (~0.5MB critical path)
                    kvt_src = kvt_d.rearrange("n p s -> p n s")
                    qt = qt_pool.tile([P, NSB, R], BF16, tag="qt", name="qt_t")
                    qt_src = qt_d[h].rearrange("n p s -> p n s")
                    kv_src = kv_d.rearrange("(n p) r -> p n r", p=P)
                    nc.gpsimd.dma_start(kvt[:, 0:1], kvt_src[:, 0:1])
                    nc.gpsimd.dma_start(qt[:, 0:1], qt_src[:, 0:1])
                    et = e_load(h, [0])
                    nc.gpsimd.dma_start(kv[:, 0:2], kv_src[:, 0:2])
                    e_load(h, [1], et=et)
                    nc.gpsimd.dma_start(kvt[:, 1:4], kvt_src[:, 1:4])
                    nc.gpsimd.dma_start(qt[:, 1:4], qt_src[:, 1:4])
                    nc.gpsimd.dma_start(kv[:, 2:NSB], kv_src[:, 2:NSB])
                    nc.gpsimd.dma_start(kvt[:, 4:NSB], kvt_src[:, 4:NSB])
                    nc.gpsimd.dma_start(qt[:, 4:NSB], qt_src[:, 4:NSB])
                    e_load(h, range(2, NSB), et=et)
                else:
                    qt = qt_load(h, nc.gpsimd)
                    et = e_load(h, range(NSB))
                return (qt, et)

            heads[0] = head_load(0, prologue=True)

            sums = sums_pool.tile([P, len(steps)], F32, tag="sums")
            carry = {}

            def stage1(i):
                h, qb = steps[i]
                nk = (qb + 1) if causal else NSB
                qt, et = heads[h]
                ps = sc_pool.tile([P, S], F32, tag="sc", name="sc_ps")
                for kb in range(nk):
                    for rb in range(NRB):
                        nc.tensor.matmul(
                            ps[:, kb * P : (kb + 1) * P],
                            kvt[:, kb, rb * P : (rb + 1) * P],
                            qt[:, qb, rb * P : (rb + 1) * P],
                            start=(rb == 0), stop=(rb == NRB - 1),
                        )
                w = nk * P
                en = en_pool.tile([P, S], BF16, tag="en", name="en_t")
                nc.scalar.activation(en[:, :w], ps[:, :w], AF.Exp, scale=SCALE)
                at = at_pool.tile([P, S], BF16, tag="at", name="at_t")
                nc.vector.tensor_mul(
                    at[:, :w].rearrange("p (n q) -> p n q", q=P),
                    en[:, :w].rearrange("p (n q) -> p n q", q=P),
                    et[:, 0:nk, qb * P : (qb + 1) * P],
                )
                carry[i] = (at, nk)

            ohs = {}

            def stage2(i):
                h, qb = steps[i]
                at, nk = carry.pop(i)
                av = av_pool.tile([P, R], F32, tag="av", name="av_ps")
                for kb in range(nk):
                    blk = at[:, kb * P : (kb + 1) * P]
                    nc.tensor.matmul(
                        av[:], blk, kv[:, kb, :],
                        start=(kb == 0), stop=(kb == nk - 1),
                    )
                    nc.tensor.matmul(
                        sums[:, i : i + 1], blk, ones[:],
                        start=(kb == 0), stop=(kb == nk - 1),
                        skip_group_check=True,
                    )
                recip = rc_pool.tile([P, 1], F32, tag="rc", name="recip_t")
                nc.vector.reciprocal(recip[:], sums[:, i : i + 1])
                g = qb // (NSB // 2)
                key = (h, g)
                if key not in ohs:
                    ohs[key] = (oh_pool.tile([P, NSB // 2, R], BF16, tag="oh",
                                             name="oh_t"), set())
                oh, done = ohs[key]
                if i % 2 == 0:
                    nc.vector.tensor_scalar_mul(oh[:, qb % 4, :], av[:], recip[:])
                else:
                    nc.scalar.activation(oh[:, qb % 4, :], av[:], AF.Copy,
                                         scale=recip[:])
                done.add(qb)
                out_ph = out_d[h].rearrange("(n p) r -> p n r", p=P)
                if h == hpc - 1 and g == 0:
                    # last head (descending): flush each row as it is ready
                    # so the final store is tiny (short pipeline tail)
                    nc.gpsimd.dma_start(out_ph[:, qb], oh[:, qb, :])
                    if qb == 0:
                        ohs.pop(key)
                elif len(done) == NSB // 2:
                    nc.gpsimd.dma_start(
                        out_ph[:, g * 4 : g * 4 + 4], oh[:]
                    )
                    ohs.pop(key)

            SKEW = 2
            for i in range(len(steps) + SKEW):
                if i < len(steps):
                    h, qb = steps[i]
                    if i % NSB == 1 and h + 1 < hpc:
                        heads[h + 1] = head_load(h + 1)
                    stage1(i)
                if i >= SKEW:
                    stage2(i - SKEW)

    if legalize:
        _legalize_sync_waits(nc)
    return nc


_CACHE = {}


def _program(causal: bool):
    if causal not in _CACHE:
        _CACHE[causal] = build_program(causal)
    return _CACHE[causal]


_TRIL = np.tril(np.ones((P, P), np.float32))


def kernel(q_nope_with_k_up, compressed_kv, rope_attention, mask,
           have_causal_mask) -> np.ndarray:
    q = np.asarray(q_nope_with_k_up, dtype=np.float32)
    kv = np.asarray(compressed_kv, dtype=np.float32)
    rope = np.asarray(rope_attention, dtype=np.float32)
    causal = bool(int(np.asarray(have_causal_mask)))

    # E = exp(scale * rope) with padding/causal masks folded in as zeros
    E = np.exp(np.float32(SCALE) * rope)
    if mask is not None:
        m = np.asarray(mask)
        if m.any():
            E *= (1.0 - m.astype(np.float32))[:, None, None, :]
    if causal:
        for qb in range(NSB):
            blk = slice(qb * P, (qb + 1) * P)
            E[:, :, blk, blk] *= _TRIL
    # transposed ([k, q] per head) bf16 copy; beyond-diagonal upper blocks
    # of the causal case are never DMA'd, so they can stay as-is
    E_T = np.ascontiguousarray(E.astype(BF16NP).transpose(0, 1, 3, 2))

    # qT/kvT shuffled to [block, p, rb*P]: qT_s[b,h,qb,p,rb*P+s'] =
    # q[b,h,qb*P+p, rb*P+s']  (r-major within each 128-row block)
    qT = np.ascontiguousarray(q.transpose(0, 1, 3, 2)).astype(BF16NP)
    qT_s = np.ascontiguousarray(
        qT.reshape(B, H, NRB, P, NSB, P).transpose(0, 1, 4, 3, 2, 5)
    ).reshape(B, H, NSB, P, R)
    kv_b = kv.astype(BF16NP)
    kvT = np.ascontiguousarray(kv_b.transpose(0, 2, 1))
    kvT_s = np.ascontiguousarray(
        kvT.reshape(B, NRB, P, NSB, P).transpose(0, 3, 2, 1, 4)
    ).reshape(B, NSB, P, R)

    nc = _program(causal)
    in_maps = []
    for c in range(N_CORES):
        b, h0 = c // (H // HPC), (c % (H // HPC)) * HPC
        in_maps.append({
            "qt": qT_s[b, h0 : h0 + HPC],
            "e": E_T[b, h0 : h0 + HPC],
            "kv": kv_b[b],
            "kvt": kvT_s[b],
        })

    res = bass_utils.run_bass_kernel_spmd(nc, in_maps, core_ids=list(range(N_CORES)))

    out = np.empty((B, H, S, R), np.float32)
    for c in range(N_CORES):
        b, h0 = c // (H // HPC), (c % (H // HPC)) * HPC
        out[b, h0 : h0 + HPC] = np.asarray(res.results[c]["out"]).astype(np.float32)
    return out


# revision 55
# speedup vs baseline: 1.0145x; 1.0145x over previous
"""MLA-style attention-score kernel for Trainium2 (8 NeuronCores, SPMD).

Computes, for full inputs
    q_nope_with_k_up [B,H,S,r], compressed_kv [B,S,r], rope_attention [B,H,S,S],
    mask [B,S], have_causal_mask scalar:

    nope   = einsum("bhqr,bkr->bhqk", q, kv)
    scores = (rope + nope) / sqrt(192)   (+ causal / padding masks)
    attn   = softmax(scores, -1)
    out    = einsum("bhqk,bkr->bhqr", attn, kv)        [B,H,S,r] fp32

Formulation: exp(s*(rope+nope)) = exp(s*rope) * exp(s*nope).  The host
precomputes E = exp(s*rope) (with causal/padding masks folded in as exact
zeros), pre-transposes q -> qT [r,S] and kv -> kvT [r,S], and casts all
operands to bf16.  On-chip, scores are built TRANSPOSED ([k,q] blocks:
stationary kvT, moving qT) so the exp output is already [k,q] and feeds the
AV matmul as stationary with no PE transpose anywhere.  Row sums come from
1-column ones-matmuls; output is scaled by 1/sum and stored bf16.

Sharding: B*H = 64 head-slots, 8 per core; core c owns batch c//2, heads
(c%2)*8..+8, with that batch's compressed_kv replicated on the core.
"""

import math
import os
import sys

import numpy as np
import ml_dtypes

for _p in ("/opt/trn_rl_repo", "/root/.axon_site/_ro/trn_rl_repo"):
    if os.path.isdir(_p) and _p not in sys.path:
        sys.path.insert(0, _p)

import concourse.bass as bass
import concourse.mybir as mybir
import concourse.tile as tile
from concourse import bass_utils
from concourse.vector_clock import ScopedClock

B, H, S, R = 4, 16, 1024, 512
N_CORES = 8
HPC = (B * H) // N_CORES          # heads per core
P = 128                           # partition block
NSB = S // P                      # 8 s-blocks
NRB = R // P                      # 4 r-chunks
SCALE = 1.0 / math.sqrt(64 + 128)
F32 = mybir.dt.float32
BF16 = mybir.dt.bfloat16
AF = mybir.ActivationFunctionType
BF16NP = ml_dtypes.bfloat16

N_WARM = int(os.environ.get("ATTN_WARM", "16"))   # PE p-state warmup matmuls


class _TC(tile.TileContext):
    """TileContext whose end-of-kernel drain splits its semaphore waits
    across preceding NOPs (walrus in this image rejects >2 sync waits on
    one Drain)."""

    MAX_WAITS = 1

    def _drain_and_barrier(self, tick_clock, wait_clock):
        nop = self.nc.sync.nop(nofuse=True)
        wait_clock.add_sem_waits(
            nop.ins, ScopedClock({None: tick_clock.global_clock})
        )
        si = nop.ins.sync_info
        waits = list(si.on_wait) if si is not None else []
        if len(waits) > self.MAX_WAITS:
            nop.ins.sync_info = mybir.SyncInfo(
                on_wait=waits[: self.MAX_WAITS], on_update=[]
            )
            for i in range(self.MAX_WAITS, len(waits), self.MAX_WAITS):
                extra = self.nc.sync.nop(nofuse=True)
                extra.ins.sync_info = mybir.SyncInfo(
                    on_wait=waits[i : i + self.MAX_WAITS], on_update=[]
                )
        self.nc.sync.drain()
        self.nc.all_engine_barrier()
        popped = self.nc._tile_sem_poison_stack.pop()
        assert popped is self._sem_poison
        # skip the hardware semaphore-clear pass + second barrier (~5us of
        # teardown): the program ends here and nothing re-reads the sems.
        # Do only the python-side bookkeeping clear_and_free_semaphores does.
        sem_nums = [
            s.num if hasattr(s, "num") else s
            for s in self.sems.allocated().values()
        ]
        if sem_nums:
            self.nc._state.prepend_free_semaphores(sem_nums)
            for poison_set in self.nc._tile_sem_poison_stack:
                poison_set.update(sem_nums)


def _legalize_sync_waits(nc, max_waits=1):
    """walrus in this image allows only `max_waits` sync waits per
    instruction: move excess waits onto NOPs inserted just before the
    instruction on the same engine queue."""
    nid = 0
    for f in nc.m.functions:
        for blk in f.blocks:
            insts = blk.instructions
            out = []
            changed = False
            for inst in insts:
                si = inst.sync_info
                if si is not None and len(si.on_wait) > max_waits:
                    waits = list(si.on_wait)
                    n_pre = len(waits) - max_waits
                    for i in range(0, n_pre, max_waits):
                        nop = mybir.InstNoOp(
                            name=f"waitsplit_{nid}", ins=[], outs=[],
                            engine=inst.engine, bass_nofuse=True,
                            sync_info=mybir.SyncInfo(
                                on_wait=waits[i : min(i + max_waits, n_pre)],
                                on_update=[],
                            ),
                        )
                        nid += 1
                        out.append(nop)
                    inst.sync_info = mybir.SyncInfo(
                        on_wait=waits[n_pre:], on_update=list(si.on_update)
                    )
                    changed = True
                out.append(inst)
            if changed:
                blk.instructions = out


def build_program(causal: bool, hpc: int = HPC, legalize: bool = True):
    # qt/kvt arrive pre-shuffled to the SBUF tile layout [block, p, rb*P]
    # so every DMA is 128 descriptors of contiguous 1KB per partition
    nc = bass.Bass()
    qt_d = nc.dram_tensor("qt", [hpc, NSB, P, R], BF16, kind="ExternalInput")
    e_d = nc.dram_tensor("e", [hpc, S, S], BF16, kind="ExternalInput")
    kv_d = nc.dram_tensor("kv", [S, R], BF16, kind="ExternalInput")
    kvt_d = nc.dram_tensor("kvt", [NSB, P, R], BF16, kind="ExternalInput")
    out_d = nc.dram_tensor("out", [hpc, S, R], BF16, kind="ExternalOutput")

    # qb ascending per head, except the LAST head runs qb descending so the
    # kernel's final step is the small nk=1 one (short pipeline tail)
    steps = []
    for h in range(hpc):
        qbs = range(NSB) if h < hpc - 1 else range(NSB - 1, -1, -1)
        steps += [(h, qb) for qb in qbs]

    with _TC(nc) as tc:
        with (
            tc.tile_pool(name="const", bufs=1) as const_pool,
            tc.tile_pool(name="kvp", bufs=1) as kv_pool,
            tc.tile_pool(name="qtp", bufs=2) as qt_pool,
            tc.tile_pool(name="ep", bufs=3) as e_pool,
            tc.tile_pool(name="enp", bufs=3) as en_pool,
            tc.tile_pool(name="atp", bufs=4) as at_pool,
            tc.tile_pool(name="ohp", bufs=4) as oh_pool,
            tc.tile_pool(name="rcp", bufs=4) as rc_pool,
            tc.tile_pool(name="psc", bufs=2, space="PSUM") as sc_pool,
            tc.tile_pool(name="pav", bufs=3, space="PSUM") as av_pool,
            tc.tile_pool(name="psum_s", bufs=1, space="PSUM") as sums_pool,
        ):
            # ---- PE warmup (p-state ramp) on memset tiles, overlapping the
            # prologue DMAs; writes a scratch psum tile nothing reads.
            # Memsets run on DVE so they don't delay the gpsimd SWDGE queue.
            wstat = const_pool.tile([P, P], BF16, tag="wstat")
            wmov = const_pool.tile([P, 512], BF16, tag="wmov")
            ones = const_pool.tile([P, 1], BF16, tag="ones")
            nc.vector.memset(wstat[:], 0.0)
            nc.vector.memset(wmov[:], 0.0)
            nc.vector.memset(ones[:], 1.0)
            warm = av_pool.tile([P, R], F32, tag="av", name="warm_ps")
            for _ in range(N_WARM):
                nc.tensor.matmul(warm[:], wstat[:], wmov[:], start=True,
                                 stop=True)

            # ---- loads. HWDGE rings generate descriptors at ~14ns/desc, so
            # big many-descriptor loads go through SWDGE (gpsimd, 0.34ns/desc):
            # prologue kvt/qt0/kv, the small-kb halves of E, and out stores.
            # Steady-state qt goes on the otherwise-idle ACT ring, the big-kb
            # halves of E on the SP ring.
            kvt = kv_pool.tile([P, NSB, R], BF16, tag="kvt")
            kv = kv_pool.tile([P, NSB, R], BF16, tag="kv")

            heads = {}

            def qt_load(h, eng):
                qt = qt_pool.tile([P, NSB, R], BF16, tag="qt", name="qt_t")
                eng.dma_start(qt[:], qt_d[h].rearrange("n p s -> p n s"))
                return qt

            def e_load(h, kbs, et=None):
                if et is None:
                    et = e_pool.tile([P, NSB, S], BF16, tag="e", name="e_t")
                for kb in kbs:
                    q0 = kb * P if causal else 0
                    eng = nc.sync if kb < NSB // 2 else nc.gpsimd
                    eng.dma_start(
                        et[:, kb, q0:S],
                        e_d[h, kb * P : (kb + 1) * P, q0:S],
                    )
                return et

            def head_load(h, prologue=False):
                if prologue:
                    # progressive block loads: step (0,0) waits only on the
                    # kb=0/qb=0 blocks (~0.5MB critical path)
                    kvt_src = kvt_d.rearrange("n p s -> p n s")
                    qt = qt_pool.tile([P, NSB, R], BF16, tag="qt", name="qt_t")
                    qt_src = qt_d[h].rearrange("n p s -> p n s")
                    kv_src = kv_d.rearrange("(n p) r -> p n r", p=P)
                    nc.gpsimd.dma_start(kvt[:, 0:1], kvt_src[:, 0:1])
                    nc.gpsimd.dma_start(qt[:, 0:1], qt_src[:, 0:1])
                    et = e_load(h, [0])
                    nc.gpsimd.dma_start(kv[:, 0:2], kv_src[:, 0:2])
                    e_load(h, [1], et=et)
                    nc.gpsimd.dma_start(kvt[:, 1:4], kvt_src[:, 1:4])
                    nc.gpsimd.dma_start(qt[:, 1:4], qt_src[:, 1:4])
                    nc.gpsimd.dma_start(kv[:, 2:NSB], kv_src[:, 2:NSB])
                    nc.gpsimd.dma_start(kvt[:, 4:NSB], kvt_src[:, 4:NSB])
                    nc.gpsimd.dma_start(qt[:, 4:NSB], qt_src[:, 4:NSB])
                    e_load(h, range(2, NSB), et=et)
                else:
                    qt = qt_load(h, nc.gpsimd)
                    et = e_load(h, range(NSB))
                return (qt, et)

            heads[0] = head_load(0, prologue=True)

            sums = sums_pool.tile([P, len(steps)], F32, tag="sums")
            carry = {}

            def stage1(i):
                h, qb = steps[i]
                nk = (qb + 1) if causal else NSB
                qt, et = heads[h]
                ps = sc_pool.tile([P, S], F32, tag="sc", name="sc_ps")
                for kb in range(nk):
                    for rb in range(NRB):
                        nc.tensor.matmul(
                            ps[:, kb * P : (kb + 1) * P],
                            kvt[:, kb, rb * P : (rb + 1) * P],
                            qt[:, qb, rb * P : (rb + 1) * P],
                            start=(rb == 0), stop=(rb == NRB - 1),
                        )
                w = nk * P
                en = en_pool.tile([P, S], BF16, tag="en", name="en_t")
                nc.scalar.activation(en[:, :w], ps[:, :w], AF.Exp, scale=SCALE)
                at = at_pool.tile([P, S], BF16, tag="at", name="at_t")
                nc.vector.tensor_mul(
                    at[:, :w].rearrange("p (n q) -> p n q", q=P),
                    en[:, :w].rearrange("p (n q) -> p n q", q=P),
                    et[:, 0:nk, qb * P : (qb + 1) * P],
                )
                carry[i] = (at, nk)

            ohs = {}

            def stage2(i):
                h, qb = steps[i]
                at, nk = carry.pop(i)
                av = av_pool.tile([P, R], F32, tag="av", name="av_ps")
                for kb in range(nk):
                    blk = at[:, kb * P : (kb + 1) * P]
                    nc.tensor.matmul(
                        av[:], blk, kv[:, kb, :],
                        start=(kb == 0), stop=(kb == nk - 1),
                    )
                    nc.tensor.matmul(
                        sums[:, i : i + 1], blk, ones[:],
                        start=(kb == 0), stop=(kb == nk - 1),
                        skip_group_check=True,
                    )
                recip = rc_pool.tile([P, 1], F32, tag="rc", name="recip_t")
                nc.vector.reciprocal(recip[:], sums[:, i : i + 1])
                g = qb // (NSB // 2)
                key = (h, g)
                if key not in ohs:
                    ohs[key] = (oh_pool.tile([P, NSB // 2, R], BF16, tag="oh",
                                             name="oh_t"), set())
                oh, done = ohs[key]
                if i % 2 == 0:
                    nc.vector.tensor_scalar_mul(oh[:, qb % 4, :], av[:], recip[:])
                else:
                    nc.scalar.activation(oh[:, qb % 4, :], av[:], AF.Copy,
                                         scale=recip[:])
                done.add(qb)
                out_ph = out_d[h].rearrange("(n p) r -> p n r", p=P)
                if h == hpc - 1 and g == 0:
                    # last head (descending): flush each row as it is ready
                    # so the final store is tiny (short pipeline tail)
                    nc.gpsimd.dma_start(out_ph[:, qb], oh[:, qb, :])
                    if qb == 0:
                        ohs.pop(key)
                elif len(done) == NSB // 2:
                    nc.gpsimd.dma_start(
                        out_ph[:, g * 4 : g * 4 + 4], oh[:]
                    )
                    ohs.pop(key)

            SKEW = 2
            for i in range(len(steps) + SKEW):
                if i < len(steps):
                    h, qb = steps[i]
                    if i % NSB == 1 and h + 1 < hpc:
                        heads[h + 1] = head_load(h + 1)
                    stage1(i)
                if i >= SKEW:
                    stage2(i - SKEW)

    if legalize:
        _legalize_sync_waits(nc)
    return nc


_CACHE = {}


def _program(causal: bool):
    if causal not in _CACHE:
        _CACHE[causal] = build_program(causal)
    return _CACHE[causal]


_TRIL = np.tril(np.ones((P, P), np.float32))


def kernel(q_nope_with_k_up, compressed_kv, rope_attention, mask,
           have_causal_mask) -> np.ndarray:
    q = np.asarray(q_nope_with_k_up, dtype=np.float32)
    kv = np.asarray(compressed_kv, dtype=np.float32)
    rope = np.asarray(rope_attention, dtype=np.float32)
    causal = bool(int(np.asarray(have_causal_mask)))

    # E = exp(scale * rope) with padding/causal masks folded in as zeros
    E = np.exp(np.float32(SCALE) * rope)
    if mask is not None:
        m = np.asarray(mask)
        if m.any():
            E *= (1.0 - m.astype(np.float32))[:, None, None, :]
    if causal:
        for qb in range(NSB):
            blk = slice(qb * P, (qb + 1) * P)
            E[:, :, blk, blk] *= _TRIL
    # transposed ([k, q] per head) bf16 copy; beyond-diagonal upper blocks
    # of the causal case are never DMA'd, so they can stay as-is
    E_T = np.ascontiguousarray(E.astype(BF16NP).transpose(0, 1, 3, 2))

    # qT/kvT shuffled to [block, p, rb*P]: qT_s[b,h,qb,p,rb*P+s'] =
    # q[b,h,qb*P+p, rb*P+s']  (r-major within each 128-row block)
    qT = np.ascontiguousarray(q.transpose(0, 1, 3, 2)).astype(BF16NP)
    qT_s = np.ascontiguousarray(
        qT.reshape(B, H, NRB, P, NSB, P).transpose(0, 1, 4, 3, 2, 5)
    ).reshape(B, H, NSB, P, R)
    kv_b = kv.astype(BF16NP)
    kvT = np.ascontiguousarray(kv_b.transpose(0, 2, 1))
    kvT_s = np.ascontiguousarray(
        kvT.reshape(B, NRB, P, NSB, P).transpose(0, 3, 2, 1, 4)
    ).reshape(B, NSB, P, R)

    nc = _program(causal)
    in_maps = []
    for c in range(N_CORES):
        b, h0 = c // (H // HPC), (c % (H // HPC)) * HPC
        in_maps.append({
            "qt": qT_s[b, h0 : h0 + HPC],
            "e": E_T[b, h0 : h0 + HPC],
            "kv": kv_b[b],
            "kvt": kvT_s[b],
        })

    res = bass_utils.run_bass_kernel_spmd(nc, in_maps, core_ids=list(range(N_CORES)))

    out = np.empty((B, H, S, R), np.float32)
    for c in range(N_CORES):
        b, h0 = c // (H // HPC), (c % (H // HPC)) * HPC
        out[b, h0 : h0 + HPC] = np.asarray(res.results[c]["out"]).astype(np.float32)
    return out


# revision 58
# speedup vs baseline: 1.0281x; 1.0134x over previous
"""MLA-style attention-score kernel for Trainium2 (8 NeuronCores, SPMD).

Computes, for full inputs
    q_nope_with_k_up [B,H,S,r], compressed_kv [B,S,r], rope_attention [B,H,S,S],
    mask [B,S], have_causal_mask scalar:

    nope   = einsum("bhqr,bkr->bhqk", q, kv)
    scores = (rope + nope) / sqrt(192)   (+ causal / padding masks)
    attn   = softmax(scores, -1)
    out    = einsum("bhqk,bkr->bhqr", attn, kv)        [B,H,S,r] fp32

Formulation: exp(s*(rope+nope)) = exp(s*rope) * exp(s*nope).  The host
precomputes E = exp(s*rope) (with causal/padding masks folded in as exact
zeros), pre-transposes q -> qT [r,S] and kv -> kvT [r,S], and casts all
operands to bf16.  On-chip, scores are built TRANSPOSED ([k,q] blocks:
stationary kvT, moving qT) so the exp output is already [k,q] and feeds the
AV matmul as stationary with no PE transpose anywhere.  Row sums come from
1-column ones-matmuls; output is scaled by 1/sum and stored bf16.

Sharding: B*H = 64 head-slots, 8 per core; core c owns batch c//2, heads
(c%2)*8..+8, with that batch's compressed_kv replicated on the core.
"""

import math
import os
import sys

import numpy as np
import ml_dtypes

for _p in ("/opt/trn_rl_repo", "/root/.axon_site/_ro/trn_rl_repo"):
    if os.path.isdir(_p) and _p not in sys.path:
        sys.path.insert(0, _p)

import concourse.bass as bass
import concourse.mybir as mybir
import concourse.tile as tile
from concourse import bass_utils
from concourse.vector_clock import ScopedClock

B, H, S, R = 4, 16, 1024, 512
N_CORES = 8
HPC = (B * H) // N_CORES          # heads per core
P = 128                           # partition block
NSB = S // P                      # 8 s-blocks
NRB = R // P                      # 4 r-chunks
SCALE = 1.0 / math.sqrt(64 + 128)
F32 = mybir.dt.float32
BF16 = mybir.dt.bfloat16
AF = mybir.ActivationFunctionType
BF16NP = ml_dtypes.bfloat16

N_WARM = int(os.environ.get("ATTN_WARM", "16"))   # PE p-state warmup matmuls


class _TC(tile.TileContext):
    """TileContext whose end-of-kernel drain splits its semaphore waits
    across preceding NOPs (walrus in this image rejects >2 sync waits on
    one Drain)."""

    MAX_WAITS = 1

    def _drain_and_barrier(self, tick_clock, wait_clock):
        nop = self.nc.sync.nop(nofuse=True)
        wait_clock.add_sem_waits(
            nop.ins, ScopedClock({None: tick_clock.global_clock})
        )
        si = nop.ins.sync_info
        waits = list(si.on_wait) if si is not None else []
        if len(waits) > self.MAX_WAITS:
            nop.ins.sync_info = mybir.SyncInfo(
                on_wait=waits[: self.MAX_WAITS], on_update=[]
            )
            for i in range(self.MAX_WAITS, len(waits), self.MAX_WAITS):
                extra = self.nc.sync.nop(nofuse=True)
                extra.ins.sync_info = mybir.SyncInfo(
                    on_wait=waits[i : i + self.MAX_WAITS], on_update=[]
                )
        self.nc.sync.drain()
        self.nc.all_engine_barrier()
        popped = self.nc._tile_sem_poison_stack.pop()
        assert popped is self._sem_poison
        # skip the hardware semaphore-clear pass + second barrier (~5us of
        # teardown): the program ends here and nothing re-reads the sems.
        # Do only the python-side bookkeeping clear_and_free_semaphores does.
        sem_nums = [
            s.num if hasattr(s, "num") else s
            for s in self.sems.allocated().values()
        ]
        if sem_nums:
            self.nc._state.prepend_free_semaphores(sem_nums)
            for poison_set in self.nc._tile_sem_poison_stack:
                poison_set.update(sem_nums)


def _legalize_sync_waits(nc, max_waits=1):
    """walrus in this image allows only `max_waits` sync waits per
    instruction: move excess waits onto NOPs inserted just before the
    instruction on the same engine queue."""
    nid = 0
    for f in nc.m.functions:
        for blk in f.blocks:
            insts = blk.instructions
            out = []
            changed = False
            for inst in insts:
                si = inst.sync_info
                if si is not None and len(si.on_wait) > max_waits:
                    waits = list(si.on_wait)
                    n_pre = len(waits) - max_waits
                    for i in range(0, n_pre, max_waits):
                        nop = mybir.InstNoOp(
                            name=f"waitsplit_{nid}", ins=[], outs=[],
                            engine=inst.engine, bass_nofuse=True,
                            sync_info=mybir.SyncInfo(
                                on_wait=waits[i : min(i + max_waits, n_pre)],
                                on_update=[],
                            ),
                        )
                        nid += 1
                        out.append(nop)
                    inst.sync_info = mybir.SyncInfo(
                        on_wait=waits[n_pre:], on_update=list(si.on_update)
                    )
                    changed = True
                out.append(inst)
            if changed:
                blk.instructions = out


def build_program(causal: bool, hpc: int = HPC, legalize: bool = True):
    # qt/kvt arrive pre-shuffled to the SBUF tile layout [block, p, rb*P]
    # so every DMA is 128 descriptors of contiguous 1KB per partition
    nc = bass.Bass()
    qt_d = nc.dram_tensor("qt", [hpc, NSB, P, R], BF16, kind="ExternalInput")
    e_d = nc.dram_tensor("e", [hpc, S, S], BF16, kind="ExternalInput")
    kv_d = nc.dram_tensor("kv", [S, R], BF16, kind="ExternalInput")
    kvt_d = nc.dram_tensor("kvt", [NSB, P, R], BF16, kind="ExternalInput")
    out_d = nc.dram_tensor("out", [hpc, S, R], BF16, kind="ExternalOutput")

    # qb ascending per head, except the LAST head runs qb descending so the
    # kernel's final step is the small nk=1 one (short pipeline tail)
    steps = []
    for h in range(hpc):
        qbs = range(NSB) if h < hpc - 1 else range(NSB - 1, -1, -1)
        steps += [(h, qb) for qb in qbs]

    with _TC(nc) as tc:
        with (
            tc.tile_pool(name="const", bufs=1) as const_pool,
            tc.tile_pool(name="kvp", bufs=1) as kv_pool,
            tc.tile_pool(name="qtp", bufs=2) as qt_pool,
            tc.tile_pool(name="ep", bufs=3) as e_pool,
            tc.tile_pool(name="enp", bufs=3) as en_pool,
            tc.tile_pool(name="atp", bufs=4) as at_pool,
            tc.tile_pool(name="ohp", bufs=4) as oh_pool,
            tc.tile_pool(name="rcp", bufs=4) as rc_pool,
            tc.tile_pool(name="psc", bufs=4, space="PSUM") as sc_pool,
            tc.tile_pool(name="pav", bufs=3, space="PSUM") as av_pool,
            tc.tile_pool(name="psum_s", bufs=1, space="PSUM") as sums_pool,
        ):
            # ---- PE warmup (p-state ramp) on memset tiles, overlapping the
            # prologue DMAs; writes a scratch psum tile nothing reads.
            # Memsets run on DVE so they don't delay the gpsimd SWDGE queue.
            wstat = const_pool.tile([P, P], BF16, tag="wstat")
            wmov = const_pool.tile([P, 512], BF16, tag="wmov")
            ones = const_pool.tile([P, 1], BF16, tag="ones")
            nc.vector.memset(wstat[:], 0.0)
            nc.vector.memset(wmov[:], 0.0)
            nc.vector.memset(ones[:], 1.0)
            warm = av_pool.tile([P, R], F32, tag="av", name="warm_ps")
            for _ in range(N_WARM):
                nc.tensor.matmul(warm[:], wstat[:], wmov[:], start=True,
                                 stop=True)

            # ---- loads. HWDGE rings generate descriptors at ~14ns/desc, so
            # big many-descriptor loads go through SWDGE (gpsimd, 0.34ns/desc):
            # prologue kvt/qt0/kv, the small-kb halves of E, and out stores.
            # Steady-state qt goes on the otherwise-idle ACT ring, the big-kb
            # halves of E on the SP ring.
            kvt = kv_pool.tile([P, NSB, R], BF16, tag="kvt")
            kv = kv_pool.tile([P, NSB, R], BF16, tag="kv")

            heads = {}

            def qt_load(h, eng):
                qt = qt_pool.tile([P, NSB, R], BF16, tag="qt", name="qt_t")
                eng.dma_start(qt[:], qt_d[h].rearrange("n p s -> p n s"))
                return qt

            def e_load(h, kbs, et=None):
                if et is None:
                    et = e_pool.tile([P, NSB, S], BF16, tag="e", name="e_t")
                for kb in kbs:
                    q0 = kb * P if causal else 0
                    eng = nc.sync if kb < NSB // 2 else nc.gpsimd
                    eng.dma_start(
                        et[:, kb, q0:S],
                        e_d[h, kb * P : (kb + 1) * P, q0:S],
                    )
                return et

            def head_load(h, prologue=False):
                if prologue:
                    # progressive block loads: step (0,0) waits only on the
                    # kb=0/qb=0 blocks (~0.5MB critical path)
                    kvt_src = kvt_d.rearrange("n p s -> p n s")
                    qt = qt_pool.tile([P, NSB, R], BF16, tag="qt", name="qt_t")
                    qt_src = qt_d[h].rearrange("n p s -> p n s")
                    kv_src = kv_d.rearrange("(n p) r -> p n r", p=P)
                    nc.gpsimd.dma_start(kvt[:, 0:1], kvt_src[:, 0:1])
                    nc.gpsimd.dma_start(qt[:, 0:1], qt_src[:, 0:1])
                    et = e_load(h, [0])
                    nc.gpsimd.dma_start(kv[:, 0:2], kv_src[:, 0:2])
                    e_load(h, [1], et=et)
                    nc.gpsimd.dma_start(kvt[:, 1:4], kvt_src[:, 1:4])
                    nc.gpsimd.dma_start(qt[:, 1:4], qt_src[:, 1:4])
                    nc.gpsimd.dma_start(kv[:, 2:NSB], kv_src[:, 2:NSB])
                    nc.gpsimd.dma_start(kvt[:, 4:NSB], kvt_src[:, 4:NSB])
                    nc.gpsimd.dma_start(qt[:, 4:NSB], qt_src[:, 4:NSB])
                    e_load(h, range(2, NSB), et=et)
                else:
                    qt = qt_load(h, nc.gpsimd)
                    et = e_load(h, range(NSB))
                return (qt, et)

            heads[0] = head_load(0, prologue=True)

            sums = sums_pool.tile([P, len(steps)], F32, tag="sums")
            carry = {}

            def stage1(i):
                h, qb = steps[i]
                nk = (qb + 1) if causal else NSB
                qt, et = heads[h]
                at = at_pool.tile([P, S], BF16, tag="at", name="at_t")
                en = en_pool.tile([P, S], BF16, tag="en", name="en_t")
                # single-bank psum chunks (bufs=4) so exp latency never
                # blocks the scores pipeline two steps ahead
                for c0 in range(0, nk, 4):
                    ncb = min(4, nk - c0)
                    w0, w = c0 * P, ncb * P
                    ps = sc_pool.tile([P, 512], F32, tag="sc", name="sc_ps")
                    for ci in range(ncb):
                        kb = c0 + ci
                        for rb in range(NRB):
                            nc.tensor.matmul(
                                ps[:, ci * P : (ci + 1) * P],
                                kvt[:, kb, rb * P : (rb + 1) * P],
                                qt[:, qb, rb * P : (rb + 1) * P],
                                start=(rb == 0), stop=(rb == NRB - 1),
                            )
                    nc.scalar.activation(en[:, w0 : w0 + w], ps[:, :w],
                                         AF.Exp, scale=SCALE)
                    nc.vector.tensor_mul(
                        at[:, w0 : w0 + w].rearrange("p (n q) -> p n q", q=P),
                        en[:, w0 : w0 + w].rearrange("p (n q) -> p n q", q=P),
                        et[:, c0 : c0 + ncb, qb * P : (qb + 1) * P],
                    )
                carry[i] = (at, nk)

            ohs = {}

            def stage2(i):
                h, qb = steps[i]
                at, nk = carry.pop(i)
                av = av_pool.tile([P, R], F32, tag="av", name="av_ps")
                for kb in range(nk):
                    blk = at[:, kb * P : (kb + 1) * P]
                    nc.tensor.matmul(
                        av[:], blk, kv[:, kb, :],
                        start=(kb == 0), stop=(kb == nk - 1),
                    )
                    nc.tensor.matmul(
                        sums[:, i : i + 1], blk, ones[:],
                        start=(kb == 0), stop=(kb == nk - 1),
                        skip_group_check=True,
                    )
                recip = rc_pool.tile([P, 1], F32, tag="rc", name="recip_t")
                nc.vector.reciprocal(recip[:], sums[:, i : i + 1])
                g = qb // (NSB // 2)
                key = (h, g)
                if key not in ohs:
                    ohs[key] = (oh_pool.tile([P, NSB // 2, R], BF16, tag="oh",
                                             name="oh_t"), set())
                oh, done = ohs[key]
                # scales live on DVE only, so ACT's queue is pure exp and
                # never delays the scores-psum handoff
                nc.vector.tensor_scalar_mul(oh[:, qb % 4, :], av[:], recip[:])
                done.add(qb)
                out_ph = out_d[h].rearrange("(n p) r -> p n r", p=P)
                if h == hpc - 1 and g == 0:
                    # last head (descending): flush each row as it is ready
                    # so the final store is tiny (short pipeline tail)
                    nc.gpsimd.dma_start(out_ph[:, qb], oh[:, qb, :])
                    if qb == 0:
                        ohs.pop(key)
                elif len(done) == NSB // 2:
                    nc.gpsimd.dma_start(
                        out_ph[:, g * 4 : g * 4 + 4], oh[:]
                    )
                    ohs.pop(key)

            SKEW = 2
            for i in range(len(steps) + SKEW):
                if i < len(steps):
                    h, qb = steps[i]
                    if i % NSB == 1 and h + 1 < hpc:
                        heads[h + 1] = head_load(h + 1)
                    stage1(i)
                if i >= SKEW:
                    stage2(i - SKEW)

    if legalize:
        _legalize_sync_waits(nc)
    return nc


_CACHE = {}


def _program(causal: bool):
    if causal not in _CACHE:
        _CACHE[causal] = build_program(causal)
    return _CACHE[causal]


_TRIL = np.tril(np.ones((P, P), np.float32))


def kernel(q_nope_with_k_up, compressed_kv, rope_attention, mask,
           have_causal_mask) -> np.ndarray:
    q = np.asarray(q_nope_with_k_up, dtype=np.float32)
    kv = np.asarray(compressed_kv, dtype=np.float32)
    rope = np.asarray(rope_attention, dtype=np.float32)
    causal = bool(int(np.asarray(have_causal_mask)))

    # E = exp(scale * rope) with padding/causal masks folded in as zeros
    E = np.exp(np.float32(SCALE) * rope)
    if mask is not None:
        m = np.asarray(mask)
        if m.any():
            E *= (1.0 - m.astype(np.float32))[:, None, None, :]
    if causal:
        for qb in range(NSB):
            blk = slice(qb * P, (qb + 1) * P)
            E[:, :, blk, blk] *= _TRIL
    # transposed ([k, q] per head) bf16 copy; beyond-diagonal upper blocks
    # of the causal case are never DMA'd, so they can stay as-is
    E_T = np.ascontiguousarray(E.astype(BF16NP).transpose(0, 1, 3, 2))

    # qT/kvT shuffled to [block, p, rb*P]: qT_s[b,h,qb,p,rb*P+s'] =
    # q[b,h,qb*P+p, rb*P+s']  (r-major within each 128-row block)
    qT = np.ascontiguousarray(q.transpose(0, 1, 3, 2)).astype(BF16NP)
    qT_s = np.ascontiguousarray(
        qT.reshape(B, H, NRB, P, NSB, P).transpose(0, 1, 4, 3, 2, 5)
    ).reshape(B, H, NSB, P, R)
    kv_b = kv.astype(BF16NP)
    kvT = np.ascontiguousarray(kv_b.transpose(0, 2, 1))
    kvT_s = np.ascontiguousarray(
        kvT.reshape(B, NRB, P, NSB, P).transpose(0, 3, 2, 1, 4)
    ).reshape(B, NSB, P, R)

    nc = _program(causal)
    in_maps = []
    for c in range(N_CORES):
        b, h0 = c // (H // HPC), (c % (H // HPC)) * HPC
        in_maps.append({
            "qt": qT_s[b, h0 : h0 + HPC],
            "e": E_T[b, h0 : h0 + HPC],
            "kv": kv_b[b],
            "kvt": kvT_s[b],
        })

    res = bass_utils.run_bass_kernel_spmd(nc, in_maps, core_ids=list(range(N_CORES)))

    out = np.empty((B, H, S, R), np.float32)
    for c in range(N_CORES):
        b, h0 = c // (H // HPC), (c % (H // HPC)) * HPC
        out[b, h0 : h0 + HPC] = np.asarray(res.results[c]["out"]).astype(np.float32)
    return out
